# revision 24
# baseline (speedup 1.0000x reference)
"""Trainium2 Bass kernel for the GNO (Galerkin-type linear attention) model.

Reference computation per batch element b (N=4096 tokens, d=64):
    h = x @ lift_w + lift_b
    for each of 4 layers:
        q = h@q_w+q_b ; k = h@k_w+k_b ; v = h@v_w+v_b
        kern     = (q @ k^T) / sqrt(d)          # [N, N], no softmax!
        integral = (kern @ v) / N               # [N, d]
        h        = gelu(h@blk_w+blk_b + integral)
    out = h @ proj_w + proj_b

Math restructure:
    (q k^T) v == q (k^T v)                    (no softmax)
    k^T v     == kvk^T G kvv,  G = H_aug^T H_aug     ([65,65] Gram)
    layer     == gelu( H @ W64 + b_eff ),  [W64; b_eff] = blkw_aug + C G kvv,
                 C = (qw_aug*s) kvk^T  (host-precomputed)

Layout: the hidden state lives as four "pair tiles" gt_p [128, 512] bf16 per
layer -- partitions 0-63 = features of token chunk p, 64-127 = features of
chunk p+4.  Each update pair is two concurrent matmuls into the two PE
column-group halves (they overlap on the PE!), one [128,512] gelu (bias adds
b_eff, replacing the ones-row trick), and NO copies back to a flat H buffer.
Transposes, the update's moving operand, and proj all read gt tiles directly
at partition base 0 or 64 (PE tile positions).  The Gram's ones column is
pre-seeded in the HT tile buffers.  All matmul operands are bf16 (4x faster
PE than fp32); accumulation stays fp32 in PSUM.  rel err ~1.5e-3 (tol 2e-2).

Sharding: batch 2 -> data-parallel on 2 NeuronCores.  Sequence-sharding
wider would need a per-layer AllReduce (>=20us latency floor) and loses.
"""

import os
import sys

for _p in ("/opt/trn_rl_repo", "/root/.axon_site/_ro/trn_rl_repo"):
    if os.path.isdir(_p) and _p not in sys.path:
        sys.path.append(_p)

import numpy as np

N = 4096          # tokens per batch element (64*64)
D = 64            # hidden
DA = D + 1        # hidden + ones row
L = 4             # layers
B = 2             # batch / cores used
SCALE = (1.0 / np.sqrt(np.float32(D))) / np.float32(N)

# wpack [65, .] column layout (65-partition constants)
DP = DA + 1  # padded HT slot stride (66) -> 4-byte aligned bf16 PSUM slots
OFF_I = 0                    # I65 (square identity)        65
OFF_KVV = OFF_I + DA         # kvv_aug per layer            4*64
OFF_CT = OFF_KVV + 4 * D     # C^T per layer                4*65
OFF_BLK = OFF_CT + 4 * DA    # blkw_aug per layer           4*64
OFF_E65 = OFF_BLK + 4 * D    # unit column e_64             1
WCOLS = OFF_E65 + 1

# wp2 [128, .] column layout (128-partition constants)
W2_I = 0                     # [I64|0 0] stacked twice      66
W2_PROJ = W2_I + DP          # [proj_w; proj_w]             1
W2_LIFT = W2_PROJ + 1        # lift_w^T in rows 0..3        64
W2COLS = W2_LIFT + D

_CACHE = {}


def _build_nc():
    """Build + compile the per-core Bass program (identical on both cores)."""
    import concourse.bass as bass
    import concourse.tile as tile
    from concourse import bacc, mybir

    f32 = mybir.dt.float32
    bf16 = mybir.dt.bfloat16
    ts = bass.ts
    GELU = mybir.ActivationFunctionType.Gelu

    nc = bacc.Bacc("TRN2", target_bir_lowering=False, debug=False, num_devices=B)

    xt_d = nc.dram_tensor("xt", [4, N], bf16, kind="ExternalInput")
    wp_d = nc.dram_tensor("wpack", [DA, WCOLS], bf16, kind="ExternalInput")
    wp2_d = nc.dram_tensor("wpack2", [128, W2COLS], bf16, kind="ExternalInput")
    y_d = nc.dram_tensor("y", [128, 32], f32, kind="ExternalOutput")

    PS = bass.MemorySpace.PSUM

    with tile.TileContext(nc) as tc:
        with (
            tc.tile_pool(name="consts", bufs=1) as consts,
            tc.tile_pool(name="htp", bufs=2) as htp,
            tc.tile_pool(name="small", bufs=2) as small,
            tc.tile_pool(name="gtp", bufs=8) as gtp,
            tc.tile_pool(name="ps_tp", bufs=2, space=PS) as ps_tp,
            tc.tile_pool(name="ps_sm", bufs=3, space=PS) as ps_sm,
            tc.tile_pool(name="ps_up", bufs=3, space=PS) as ps_up,
        ):
            # ---- load inputs into SBUF (parallel DMA queues) -------------
            xt = consts.tile([4, N], bf16, tag="xt")
            nc.sync.dma_start(xt[:], xt_d.ap())
            wp = consts.tile([DA, WCOLS], bf16, tag="wp")
            nc.gpsimd.dma_start(wp[:], wp_d.ap())
            wp2 = consts.tile([128, W2COLS], bf16, tag="wp2")
            nc.scalar.dma_start(wp2[:], wp2_d.ap())

            I65 = wp[:, OFF_I : OFF_I + DA]
            e65 = wp[:, OFF_E65 : OFF_E65 + 1]

            # HT buffers: 32 slots of stride 66 (cols 0-63 = transposed
            # tokens x features, col 64 = ones, col 65 = pad).  The ones
            # columns are seeded once per buffer and never overwritten.
            HTb = []
            for hb in range(2):
                HT = htp.tile([128, 32 * DP], bf16, tag="ht", name=f"ht{hb}")
                nc.vector.memset(
                    HT.rearrange("p (t c) -> p t c", c=DP)[:, :, D : D + 1],
                    1.0)
                HTb.append(HT)

            # ---- lift: pair tiles gt_c = lift_w^T @ [xt_c | xt_{c+4}] ----
            state = []
            for c in range(4):
                lf_ps = ps_up.tile([128, 512], f32, tag="up")
                nc.tensor.matmul(lf_ps[0:D, :], wp2[0:4, W2_LIFT : W2_LIFT + D],
                                 xt[:, ts(c, 512)], start=True, stop=True)
                nc.tensor.matmul(lf_ps[64:128, :], wp2[0:4, W2_LIFT : W2_LIFT + D],
                                 xt[:, ts(c + 4, 512)], start=True, stop=True)
                gt = gtp.tile([128, 512], bf16, tag="gt", name=f"lift{c}")
                if c % 2 == 0:
                    nc.vector.tensor_copy(gt[:], lf_ps[:])
                else:
                    nc.scalar.copy(gt[:], lf_ps[:])
                state.append(gt)

            # ---- layers --------------------------------------------------
            for l in range(L):
                kvv_l = wp[:, OFF_KVV + l * D : OFF_KVV + (l + 1) * D]
                ct_l = wp[:, OFF_CT + l * DA : OFF_CT + (l + 1) * DA]
                blk_l = wp[:, OFF_BLK + l * D : OFF_BLK + (l + 1) * D]
                HT = HTb[l % 2]

                # W64 accumulator [128,64] (W64 duplicated in both halves):
                # starts with blkw rows 0-63, chain below adds (C G kvv).
                w_ps = ps_sm.tile([128, D], f32, tag="sm")
                nc.tensor.matmul(w_ps[0:D, :], I65[:, 0:D], blk_l,
                                 start=True, stop=False)
                nc.tensor.matmul(w_ps[64:128, :], I65[:, 0:D], blk_l,
                                 start=True, stop=False)

                g_ps = ps_sm.tile([DA, DA], f32, tag="sm")

                # transposes: tile t (tokens 128t..) lives in pair tile
                # state[c%4], partition half c//4, where c = t//4.
                def transpose_group(g):
                    tp_ps = ps_tp.tile([128, 4 * DP], bf16, tag="tp")
                    for k in range(4):
                        t = 4 * g + k
                        c = t // 4
                        src = state[c % 4]
                        rb = 64 * (c // 4)
                        nc.tensor.transpose(
                            tp_ps[:, k * DP : k * DP + DP],
                            src[rb : rb + D,
                                128 * (t % 4) : 128 * (t % 4) + 128],
                            wp2[rb : rb + D, W2_I : W2_I + DP])
                    # strided copy: per-slot cols 0-63 only (ones col
                    # stays); alternate vector/scalar to halve the phase
                    # floor (both engines are otherwise idle here)
                    dst = HT.rearrange("p (t c) -> p t c", c=DP)\
                            [:, 4 * g : 4 * g + 4, 0:D]
                    srcp = tp_ps.rearrange("p (t c) -> p t c", c=DP)[:, :, 0:D]
                    if g % 2 == 0:
                        nc.vector.tensor_copy(dst, srcp)
                    else:
                        nc.scalar.copy(dst, srcp)

                def g_group(g):
                    for k in range(4):
                        t = 4 * g + k
                        nc.tensor.matmul(
                            g_ps[:], HT[:, t * DP : t * DP + DA],
                            HT[:, t * DP : t * DP + DA],
                            start=(t == 0), stop=(t == 31))

                # Gram groups lag the transposes by 2 so the PSUM->SBUF
                # copies have a full group-time to land.
                for g in range(8):
                    transpose_group(g)
                    if g > 1:
                        g_group(g - 2)
                g_group(6)
                g_group(7)

                # chain: [W64; b_eff] += C @ (G @ kvv)
                g_sb = small.tile([DA, DA], bf16, tag="gsb")
                nc.vector.tensor_copy(g_sb[:], g_ps[:])
                m1_ps = ps_sm.tile([DA, D], f32, tag="sm")
                nc.tensor.matmul(m1_ps[:], g_sb[:], kvv_l, start=True, stop=True)
                m1_sb = small.tile([DA, D], bf16, tag="m1")
                nc.vector.tensor_copy(m1_sb[:], m1_ps[:])
                nc.tensor.matmul(w_ps[0:D, :], ct_l[:, 0:D], m1_sb[:],
                                 start=False, stop=True)
                nc.tensor.matmul(w_ps[64:128, :], ct_l[:, 0:D], m1_sb[:],
                                 start=False, stop=True)
                wupd = small.tile([128, D], bf16, tag="wupd")
                nc.vector.tensor_copy(wupd[:], w_ps[:])
                # bias b_eff = row 64 of (blkw + C G kvv), both halves
                b_ps = ps_sm.tile([128, 1], f32, tag="sm")
                nc.tensor.matmul(b_ps[0:D, :], blk_l, e65,
                                 start=True, stop=False)
                nc.tensor.matmul(b_ps[64:128, :], blk_l, e65,
                                 start=True, stop=False)
                nc.tensor.matmul(b_ps[0:D, :], m1_sb[:], ct_l[:, D : DA],
                                 start=False, stop=True)
                nc.tensor.matmul(b_ps[64:128, :], m1_sb[:], ct_l[:, D : DA],
                                 start=False, stop=True)
                bias = small.tile([128, 1], f32, tag="bias")
                nc.vector.tensor_copy(bias[:], b_ps[:])

                # update: gt'_p = gelu([h_p | h_{p+4}] @ W64 + b_eff); the
                # two matmuls run concurrently in disjoint PE quadrants.
                if l == L - 1:
                    y_ps = ps_sm.tile([128, 32], f32, tag="sm")
                nstate = []
                for p in range(4):
                    up_ps = ps_up.tile([128, 512], f32, tag="up")
                    nc.tensor.matmul(up_ps[0:D, :], wupd[0:D, :],
                                     state[p][0:D, :], start=True, stop=True)
                    nc.tensor.matmul(up_ps[64:128, :], wupd[64:128, :],
                                     state[p][64:128, :], start=True, stop=True)
                    gt = gtp.tile([128, 512], bf16, tag="gt", name=f"gt{l}_{p}")
                    nc.scalar.activation(gt[:], up_ps[:], GELU, bias=bias[:])
                    nstate.append(gt)
                    if l == L - 1:
                        # proj (token-major, y_ps[q, t] = y[128*t + q]);
                        # proj_b is added on the host after gather.
                        for half in range(2):
                            c = p + 4 * half
                            rb = 64 * half
                            for s in range(4):
                                tt = 4 * c + s
                                nc.tensor.matmul(
                                    y_ps[:, tt : tt + 1],
                                    gt[rb : rb + D, 128 * s : 128 * s + 128],
                                    wp2[rb : rb + D, W2_PROJ : W2_PROJ + 1],
                                    start=True, stop=True)
                state = nstate

            # ---- drain y -------------------------------------------------
            out_sb = consts.tile([128, 32], f32, tag="out")
            nc.vector.tensor_copy(out_sb[:], y_ps[:])
            nc.sync.dma_start(y_d.ap(), out_sb[:])

    nc.compile()
    return nc


def _prep_inputs(x, lift_w, lift_b, blk_w, blk_b, q_w, q_b, k_w, k_b, v_w,
                 v_b, proj_w, proj_b):
    """Host-side weight packing (tiny [64,64] reshuffles, negligible cost)."""
    import ml_dtypes
    bf16 = ml_dtypes.bfloat16
    f = lambda a: np.asarray(a, dtype=np.float32)
    x = f(x)
    lift_w, lift_b = f(lift_w), f(lift_b)
    blk_w, blk_b = f(blk_w), f(blk_b)
    q_w, q_b, k_w, k_b, v_w, v_b = f(q_w), f(q_b), f(k_w), f(k_b), f(v_w), f(v_b)
    proj_w, proj_b = f(proj_w), f(proj_b)

    wpack = np.zeros((DA, WCOLS), np.float32)
    wpack[:, OFF_I : OFF_I + DA] = np.eye(DA, dtype=np.float32)
    for l in range(L):
        kvk = np.vstack([k_w[l], k_b[l][None]])            # [65, 64]
        kvv = np.vstack([v_w[l], v_b[l][None]])            # [65, 64]
        qts = (np.vstack([q_w[l], q_b[l][None]]) * SCALE).T  # [64, 65]
        ct = kvk @ qts                                     # [65, 65] = C^T
        wpack[:, OFF_KVV + l * D : OFF_KVV + (l + 1) * D] = kvv
        wpack[:, OFF_CT + l * DA : OFF_CT + (l + 1) * DA] = ct
        wpack[:, OFF_BLK + l * D : OFF_BLK + (l + 1) * D] = \
            np.vstack([blk_w[l], blk_b[l][None]])
    wpack[D, OFF_E65] = 1.0
    wpack = wpack.astype(bf16)

    wpack2 = np.zeros((128, W2COLS), np.float32)
    wpack2[0:D, W2_I : W2_I + D] = np.eye(D, dtype=np.float32)
    wpack2[D : 2 * D, W2_I : W2_I + D] = np.eye(D, dtype=np.float32)
    wpack2[0:D, W2_PROJ] = proj_w[:, 0]
    wpack2[D : 2 * D, W2_PROJ] = proj_w[:, 0]
    wpack2[0:3, W2_LIFT : W2_LIFT + D] = lift_w
    wpack2[3, W2_LIFT : W2_LIFT + D] = lift_b
    wpack2 = wpack2.astype(bf16)

    in_maps = []
    for b in range(B):
        xt = np.concatenate([x[b].reshape(N, 3).T,
                             np.ones((1, N), np.float32)], axis=0).astype(bf16)
        in_maps.append({"xt": np.ascontiguousarray(xt), "wpack": wpack,
                        "wpack2": wpack2})
    return in_maps, x.shape, np.float32(proj_b[0])


def _get_runner():
    """Compile once, return a fn(in_maps) -> list[{name: np.ndarray}]."""
    if "runner" in _CACHE:
        return _CACHE["runner"]

    import jax
    from jax.sharding import Mesh, PartitionSpec
    try:
        from jax.experimental.shard_map import shard_map
    except ImportError:  # newer jax
        from jax.sharding import shard_map
    from concourse import mybir
    from concourse.bass2jax import (_bass_exec_p, install_neuronx_cc_hook,
                                    partition_id_tensor)

    nc = _build_nc()
    install_neuronx_cc_hook()

    partition_name = (nc.partition_id_tensor.name
                      if nc.partition_id_tensor else None)
    in_names, out_names, out_avals, zero_outs = [], [], [], []
    for alloc in nc.m.functions[0].allocations:
        if not isinstance(alloc, mybir.MemoryLocationSet):
            continue
        name = alloc.memorylocations[0].name
        if alloc.kind == "ExternalInput":
            if name != partition_name:
                in_names.append(name)
        elif alloc.kind == "ExternalOutput":
            shape = tuple(alloc.tensor_shape)
            dtype = mybir.dt.np(alloc.dtype)
            out_names.append(name)
            out_avals.append(jax.core.ShapedArray(shape, dtype))
            zero_outs.append(np.zeros(shape, dtype))
    n_params = len(in_names)
    n_outs = len(out_avals)
    all_in_names = in_names + out_names + ([partition_name] if partition_name else [])
    donate = tuple(range(n_params, n_params + n_outs))

    def _body(*args):
        operands = list(args)
        if partition_name is not None:
            operands.append(partition_id_tensor())
        return tuple(_bass_exec_p.bind(
            *operands, out_avals=tuple(out_avals), in_names=tuple(all_in_names),
            out_names=tuple(out_names), lowering_input_output_aliases=(),
            sim_require_finite=True, sim_require_nnan=True, nc=nc))

    devices = jax.devices()[:B]
    mesh = Mesh(np.asarray(devices), ("core",))
    sharded = jax.jit(
        shard_map(_body, mesh=mesh,
                  in_specs=(PartitionSpec("core"),) * (n_params + n_outs),
                  out_specs=(PartitionSpec("core"),) * n_outs,
                  check_rep=False),
        donate_argnums=donate, keep_unused=True)

    def run(in_maps):
        per_core = [[np.asarray(m[name]) for name in in_names] for m in in_maps]
        concat_in = [np.concatenate([per_core[c][i] for c in range(B)], axis=0)
                     for i in range(n_params)]
        big_zeros = [np.concatenate([z] * B, axis=0) for z in zero_outs]
        outs = jax.block_until_ready(sharded(*concat_in, *big_zeros))
        results = []
        for c in range(B):
            r = {}
            for i, name in enumerate(out_names):
                rows = out_avals[i].shape[0]
                r[name] = np.asarray(outs[i][c * rows : (c + 1) * rows])
            results.append(r)
        return results

    _CACHE["runner"] = run
    return run


def kernel(**inputs) -> np.ndarray:
    in_maps, x_shape, proj_b0 = _prep_inputs(**inputs)
    run = _get_runner()
    results = run(in_maps)
    # y_core [128, 32]: element (q, t) = y[128*t + q] (pre proj_b)
    out = np.stack([(results[b]["y"].T + proj_b0)
                    .reshape(x_shape[1], x_shape[2], 1) for b in range(B)])
    return out.astype(np.float32)


# revision 26
# speedup vs baseline: 1.0139x; 1.0139x over previous
"""Trainium2 Bass kernel for the GNO (Galerkin-type linear attention) model.

Reference computation per batch element b (N=4096 tokens, d=64):
    h = x @ lift_w + lift_b
    for each of 4 layers:
        q = h@q_w+q_b ; k = h@k_w+k_b ; v = h@v_w+v_b
        kern     = (q @ k^T) / sqrt(d)          # [N, N], no softmax!
        integral = (kern @ v) / N               # [N, d]
        h        = gelu(h@blk_w+blk_b + integral)
    out = h @ proj_w + proj_b

Math restructure:
    (q k^T) v == q (k^T v)                    (no softmax)
    k^T v     == kvk^T G kvv,  G = H_aug^T H_aug     ([65,65] Gram)
    layer     == gelu( H @ W64 + b_eff ),  [W64; b_eff] = blkw_aug + C G kvv,
                 C = (qw_aug*s) kvk^T  (host-precomputed)

Layout: the hidden state lives as four "pair tiles" gt_p [128, 512] bf16 per
layer -- partitions 0-63 = features of token chunk p, 64-127 = features of
chunk p+4.  Each update pair is two concurrent matmuls into the two PE
column-group halves (they overlap on the PE!), one [128,512] gelu (bias adds
b_eff, replacing the ones-row trick), and NO copies back to a flat H buffer.
Transposes, the update's moving operand, and proj all read gt tiles directly
at partition base 0 or 64 (PE tile positions).  The Gram's ones column is
pre-seeded in the HT tile buffers.  All matmul operands are bf16 (4x faster
PE than fp32); accumulation stays fp32 in PSUM.  rel err ~1.5e-3 (tol 2e-2).

Sharding: batch 2 -> data-parallel on 2 NeuronCores.  Sequence-sharding
wider would need a per-layer AllReduce (>=20us latency floor) and loses.
"""

import os
import sys

for _p in ("/opt/trn_rl_repo", "/root/.axon_site/_ro/trn_rl_repo"):
    if os.path.isdir(_p) and _p not in sys.path:
        sys.path.append(_p)

import numpy as np

N = 4096          # tokens per batch element (64*64)
D = 64            # hidden
DA = D + 1        # hidden + ones row
L = 4             # layers
B = 2             # batch / cores used
SCALE = (1.0 / np.sqrt(np.float32(D))) / np.float32(N)

# wpack [65, .] column layout (65-partition constants)
DP = DA + 1  # padded HT slot stride (66) -> 4-byte aligned bf16 PSUM slots
OFF_I = 0                    # I65 (square identity)        65
OFF_KVV = OFF_I + DA         # kvv_aug per layer            4*64
OFF_CT = OFF_KVV + 4 * D     # C^T per layer                4*65
OFF_BLK = OFF_CT + 4 * DA    # blkw_aug per layer           4*64
OFF_E65 = OFF_BLK + 4 * D    # unit column e_64             1
WCOLS = OFF_E65 + 1

# wp2 [128, .] column layout (128-partition constants)
W2_I = 0                     # [I64|0 0] stacked twice      66
W2_PROJ = W2_I + DP          # [proj_w; proj_w]             1
W2_LIFT = W2_PROJ + 1        # lift_w^T in rows 0..3        64
W2COLS = W2_LIFT + D

_CACHE = {}


def _build_nc():
    """Build + compile the per-core Bass program (identical on both cores)."""
    import concourse.bass as bass
    import concourse.tile as tile
    from concourse import bacc, mybir

    f32 = mybir.dt.float32
    bf16 = mybir.dt.bfloat16
    ts = bass.ts
    GELU = mybir.ActivationFunctionType.Gelu

    nc = bacc.Bacc("TRN2", target_bir_lowering=False, debug=False, num_devices=B)

    xt_d = nc.dram_tensor("xt", [4, N], bf16, kind="ExternalInput")
    wp_d = nc.dram_tensor("wpack", [DA, WCOLS], bf16, kind="ExternalInput")
    wp2_d = nc.dram_tensor("wpack2", [128, W2COLS], bf16, kind="ExternalInput")
    y_d = nc.dram_tensor("y", [128, 32], f32, kind="ExternalOutput")

    PS = bass.MemorySpace.PSUM

    with tile.TileContext(nc) as tc:
        with (
            tc.tile_pool(name="consts", bufs=1) as consts,
            tc.tile_pool(name="htp", bufs=2) as htp,
            tc.tile_pool(name="small", bufs=2) as small,
            tc.tile_pool(name="gtp", bufs=8) as gtp,
            tc.tile_pool(name="ps_tp", bufs=2, space=PS) as ps_tp,
            tc.tile_pool(name="ps_sm", bufs=3, space=PS) as ps_sm,
            tc.tile_pool(name="ps_up", bufs=3, space=PS) as ps_up,
        ):
            # ---- load inputs into SBUF (parallel DMA queues) -------------
            xt = consts.tile([4, N], bf16, tag="xt")
            nc.sync.dma_start(xt[:], xt_d.ap())
            wp = consts.tile([DA, WCOLS], bf16, tag="wp")
            nc.gpsimd.dma_start(wp[:], wp_d.ap())
            wp2 = consts.tile([128, W2COLS], bf16, tag="wp2")
            nc.scalar.dma_start(wp2[:], wp2_d.ap())

            I65 = wp[:, OFF_I : OFF_I + DA]
            e65 = wp[:, OFF_E65 : OFF_E65 + 1]

            # HT buffers: 32 slots of stride 66 (cols 0-63 = transposed
            # tokens x features, col 64 = ones, col 65 = pad).  The ones
            # columns are seeded once per buffer and never overwritten.
            HTb = []
            for hb in range(2):
                HT = htp.tile([128, 32 * DP], bf16, tag="ht", name=f"ht{hb}")
                nc.vector.memset(
                    HT.rearrange("p (t c) -> p t c", c=DP)[:, :, D : D + 1],
                    1.0)
                HTb.append(HT)

            # ---- lift: pair tiles gt_c = lift_w^T @ [xt_c | xt_{c+4}] ----
            state = []
            for c in range(4):
                lf_ps = ps_up.tile([128, 512], f32, tag="up")
                nc.tensor.matmul(lf_ps[0:D, :], wp2[0:4, W2_LIFT : W2_LIFT + D],
                                 xt[:, ts(c, 512)], start=True, stop=True)
                nc.tensor.matmul(lf_ps[64:128, :], wp2[0:4, W2_LIFT : W2_LIFT + D],
                                 xt[:, ts(c + 4, 512)], start=True, stop=True)
                gt = gtp.tile([128, 512], bf16, tag="gt", name=f"lift{c}")
                if c % 2 == 0:
                    nc.vector.tensor_copy(gt[:], lf_ps[:])
                else:
                    nc.scalar.copy(gt[:], lf_ps[:])
                state.append(gt)

            # transposes: tile t (tokens 128t..) of layer lx lives in pair
            # tile st[c%4], partition half c//4, where c = t//4.
            def transpose_group(lx, g, st):
                HTx = HTb[lx % 2]
                tp_ps = ps_tp.tile([128, 4 * DP], bf16, tag="tp",
                                   name=f"tp{lx}_{g}")
                for k in range(4):
                    t = 4 * g + k
                    c = t // 4
                    rb = 64 * (c // 4)
                    nc.tensor.transpose(
                        tp_ps[:, k * DP : k * DP + DP],
                        st[c % 4][rb : rb + D,
                                  128 * (t % 4) : 128 * (t % 4) + 128],
                        wp2[rb : rb + D, W2_I : W2_I + DP])
                # strided copy: per-slot cols 0-63 only (ones col stays)
                nc.vector.tensor_copy(
                    HTx.rearrange("p (t c) -> p t c", c=DP)
                       [:, 4 * g : 4 * g + 4, 0:D],
                    tp_ps.rearrange("p (t c) -> p t c", c=DP)[:, :, 0:D])

            def hoisted_transpose(lx, g, st):
                transpose_group(lx, g, st)

            # ---- layers --------------------------------------------------
            for l in range(L):
                kvv_l = wp[:, OFF_KVV + l * D : OFF_KVV + (l + 1) * D]
                ct_l = wp[:, OFF_CT + l * DA : OFF_CT + (l + 1) * DA]
                blk_l = wp[:, OFF_BLK + l * D : OFF_BLK + (l + 1) * D]
                HT = HTb[l % 2]

                # W64 accumulator [128,64] (W64 duplicated in both halves):
                # starts with blkw rows 0-63, chain below adds (C G kvv).
                w_ps = ps_sm.tile([128, D], f32, tag="sm")
                nc.tensor.matmul(w_ps[0:D, :], I65[:, 0:D], blk_l,
                                 start=True, stop=False)
                nc.tensor.matmul(w_ps[64:128, :], I65[:, 0:D], blk_l,
                                 start=True, stop=False)

                g_ps = ps_sm.tile([DA, DA], f32, tag="sm")

                def g_group(g):
                    for k in range(4):
                        t = 4 * g + k
                        nc.tensor.matmul(
                            g_ps[:], HT[:, t * DP : t * DP + DA],
                            HT[:, t * DP : t * DP + DA],
                            start=(t == 0), stop=(t == 31))

                # Gram groups lag the transposes so the PSUM->SBUF copies
                # have time to land.  Transpose groups 0-3 were hoisted
                # into the previous layer's update loop (except layer 0).
                if l == 0:
                    for g in range(8):
                        transpose_group(l, g, state)
                        if g > 1:
                            g_group(g - 2)
                    g_group(6)
                    g_group(7)
                else:
                    for g in range(4, 8):
                        transpose_group(l, g, state)
                        g_group(g - 4)
                    for g in range(4, 8):
                        g_group(g)

                # chain: [W64; b_eff] += C @ (G @ kvv)
                g_sb = small.tile([DA, DA], bf16, tag="gsb")
                nc.vector.tensor_copy(g_sb[:], g_ps[:])
                m1_ps = ps_sm.tile([DA, D], f32, tag="sm")
                nc.tensor.matmul(m1_ps[:], g_sb[:], kvv_l, start=True, stop=True)
                m1_sb = small.tile([DA, D], bf16, tag="m1")
                nc.vector.tensor_copy(m1_sb[:], m1_ps[:])
                nc.tensor.matmul(w_ps[0:D, :], ct_l[:, 0:D], m1_sb[:],
                                 start=False, stop=True)
                nc.tensor.matmul(w_ps[64:128, :], ct_l[:, 0:D], m1_sb[:],
                                 start=False, stop=True)
                wupd = small.tile([128, D], bf16, tag="wupd")
                nc.vector.tensor_copy(wupd[:], w_ps[:])
                # bias b_eff = row 64 of (blkw + C G kvv), both halves
                b_ps = ps_sm.tile([128, 1], f32, tag="sm")
                nc.tensor.matmul(b_ps[0:D, :], blk_l, e65,
                                 start=True, stop=False)
                nc.tensor.matmul(b_ps[64:128, :], blk_l, e65,
                                 start=True, stop=False)
                nc.tensor.matmul(b_ps[0:D, :], m1_sb[:], ct_l[:, D : DA],
                                 start=False, stop=True)
                nc.tensor.matmul(b_ps[64:128, :], m1_sb[:], ct_l[:, D : DA],
                                 start=False, stop=True)
                bias = small.tile([128, 1], f32, tag="bias")
                nc.vector.tensor_copy(bias[:], b_ps[:])

                # update: gt'_p = gelu([h_p | h_{p+4}] @ W64 + b_eff); the
                # two matmuls run concurrently in disjoint PE quadrants.
                # The NEXT layer's transpose group p is emitted right after
                # gelu p, filling the tensor engine during the gelu tail.
                if l == L - 1:
                    y_ps = ps_sm.tile([128, 32], f32, tag="sm")
                nstate = []
                for p in range(4):
                    up_ps = ps_up.tile([128, 512], f32, tag="up")
                    nc.tensor.matmul(up_ps[0:D, :], wupd[0:D, :],
                                     state[p][0:D, :], start=True, stop=True)
                    nc.tensor.matmul(up_ps[64:128, :], wupd[64:128, :],
                                     state[p][64:128, :], start=True, stop=True)
                    gt = gtp.tile([128, 512], bf16, tag="gt", name=f"gt{l}_{p}")
                    nc.scalar.activation(gt[:], up_ps[:], GELU, bias=bias[:])
                    nstate.append(gt)
                    if l == L - 1:
                        # proj (token-major, y_ps[q, t] = y[128*t + q]);
                        # proj_b is added on the host after gather.
                        for half in range(2):
                            c = p + 4 * half
                            rb = 64 * half
                            for s in range(4):
                                tt = 4 * c + s
                                nc.tensor.matmul(
                                    y_ps[:, tt : tt + 1],
                                    gt[rb : rb + D, 128 * s : 128 * s + 128],
                                    wp2[rb : rb + D, W2_PROJ : W2_PROJ + 1],
                                    start=True, stop=True)
                    else:
                        hoisted_transpose(l + 1, p, nstate)
                state = nstate

            # ---- drain y -------------------------------------------------
            out_sb = consts.tile([128, 32], f32, tag="out")
            nc.vector.tensor_copy(out_sb[:], y_ps[:])
            nc.sync.dma_start(y_d.ap(), out_sb[:])

    nc.compile()
    return nc


def _prep_inputs(x, lift_w, lift_b, blk_w, blk_b, q_w, q_b, k_w, k_b, v_w,
                 v_b, proj_w, proj_b):
    """Host-side weight packing (tiny [64,64] reshuffles, negligible cost)."""
    import ml_dtypes
    bf16 = ml_dtypes.bfloat16
    f = lambda a: np.asarray(a, dtype=np.float32)
    x = f(x)
    lift_w, lift_b = f(lift_w), f(lift_b)
    blk_w, blk_b = f(blk_w), f(blk_b)
    q_w, q_b, k_w, k_b, v_w, v_b = f(q_w), f(q_b), f(k_w), f(k_b), f(v_w), f(v_b)
    proj_w, proj_b = f(proj_w), f(proj_b)

    wpack = np.zeros((DA, WCOLS), np.float32)
    wpack[:, OFF_I : OFF_I + DA] = np.eye(DA, dtype=np.float32)
    for l in range(L):
        kvk = np.vstack([k_w[l], k_b[l][None]])            # [65, 64]
        kvv = np.vstack([v_w[l], v_b[l][None]])            # [65, 64]
        qts = (np.vstack([q_w[l], q_b[l][None]]) * SCALE).T  # [64, 65]
        ct = kvk @ qts                                     # [65, 65] = C^T
        wpack[:, OFF_KVV + l * D : OFF_KVV + (l + 1) * D] = kvv
        wpack[:, OFF_CT + l * DA : OFF_CT + (l + 1) * DA] = ct
        wpack[:, OFF_BLK + l * D : OFF_BLK + (l + 1) * D] = \
            np.vstack([blk_w[l], blk_b[l][None]])
    wpack[D, OFF_E65] = 1.0
    wpack = wpack.astype(bf16)

    wpack2 = np.zeros((128, W2COLS), np.float32)
    wpack2[0:D, W2_I : W2_I + D] = np.eye(D, dtype=np.float32)
    wpack2[D : 2 * D, W2_I : W2_I + D] = np.eye(D, dtype=np.float32)
    wpack2[0:D, W2_PROJ] = proj_w[:, 0]
    wpack2[D : 2 * D, W2_PROJ] = proj_w[:, 0]
    wpack2[0:3, W2_LIFT : W2_LIFT + D] = lift_w
    wpack2[3, W2_LIFT : W2_LIFT + D] = lift_b
    wpack2 = wpack2.astype(bf16)

    in_maps = []
    for b in range(B):
        xt = np.concatenate([x[b].reshape(N, 3).T,
                             np.ones((1, N), np.float32)], axis=0).astype(bf16)
        in_maps.append({"xt": np.ascontiguousarray(xt), "wpack": wpack,
                        "wpack2": wpack2})
    return in_maps, x.shape, np.float32(proj_b[0])


def _get_runner():
    """Compile once, return a fn(in_maps) -> list[{name: np.ndarray}]."""
    if "runner" in _CACHE:
        return _CACHE["runner"]

    import jax
    from jax.sharding import Mesh, PartitionSpec
    try:
        from jax.experimental.shard_map import shard_map
    except ImportError:  # newer jax
        from jax.sharding import shard_map
    from concourse import mybir
    from concourse.bass2jax import (_bass_exec_p, install_neuronx_cc_hook,
                                    partition_id_tensor)

    nc = _build_nc()
    install_neuronx_cc_hook()

    partition_name = (nc.partition_id_tensor.name
                      if nc.partition_id_tensor else None)
    in_names, out_names, out_avals, zero_outs = [], [], [], []
    for alloc in nc.m.functions[0].allocations:
        if not isinstance(alloc, mybir.MemoryLocationSet):
            continue
        name = alloc.memorylocations[0].name
        if alloc.kind == "ExternalInput":
            if name != partition_name:
                in_names.append(name)
        elif alloc.kind == "ExternalOutput":
            shape = tuple(alloc.tensor_shape)
            dtype = mybir.dt.np(alloc.dtype)
            out_names.append(name)
            out_avals.append(jax.core.ShapedArray(shape, dtype))
            zero_outs.append(np.zeros(shape, dtype))
    n_params = len(in_names)
    n_outs = len(out_avals)
    all_in_names = in_names + out_names + ([partition_name] if partition_name else [])
    donate = tuple(range(n_params, n_params + n_outs))

    def _body(*args):
        operands = list(args)
        if partition_name is not None:
            operands.append(partition_id_tensor())
        return tuple(_bass_exec_p.bind(
            *operands, out_avals=tuple(out_avals), in_names=tuple(all_in_names),
            out_names=tuple(out_names), lowering_input_output_aliases=(),
            sim_require_finite=True, sim_require_nnan=True, nc=nc))

    devices = jax.devices()[:B]
    mesh = Mesh(np.asarray(devices), ("core",))
    sharded = jax.jit(
        shard_map(_body, mesh=mesh,
                  in_specs=(PartitionSpec("core"),) * (n_params + n_outs),
                  out_specs=(PartitionSpec("core"),) * n_outs,
                  check_rep=False),
        donate_argnums=donate, keep_unused=True)

    def run(in_maps):
        per_core = [[np.asarray(m[name]) for name in in_names] for m in in_maps]
        concat_in = [np.concatenate([per_core[c][i] for c in range(B)], axis=0)
                     for i in range(n_params)]
        big_zeros = [np.concatenate([z] * B, axis=0) for z in zero_outs]
        outs = jax.block_until_ready(sharded(*concat_in, *big_zeros))
        results = []
        for c in range(B):
            r = {}
            for i, name in enumerate(out_names):
                rows = out_avals[i].shape[0]
                r[name] = np.asarray(outs[i][c * rows : (c + 1) * rows])
            results.append(r)
        return results

    _CACHE["runner"] = run
    return run


def kernel(**inputs) -> np.ndarray:
    in_maps, x_shape, proj_b0 = _prep_inputs(**inputs)
    run = _get_runner()
    results = run(in_maps)
    # y_core [128, 32]: element (q, t) = y[128*t + q] (pre proj_b)
    out = np.stack([(results[b]["y"].T + proj_b0)
                    .reshape(x_shape[1], x_shape[2], 1) for b in range(B)])
    return out.astype(np.float32)


# revision 27
# speedup vs baseline: 1.0464x; 1.0321x over previous
"""Trainium2 Bass kernel for the GNO (Galerkin-type linear attention) model.

Reference computation per batch element b (N=4096 tokens, d=64):
    h = x @ lift_w + lift_b
    for each of 4 layers:
        q = h@q_w+q_b ; k = h@k_w+k_b ; v = h@v_w+v_b
        kern     = (q @ k^T) / sqrt(d)          # [N, N], no softmax!
        integral = (kern @ v) / N               # [N, d]
        h        = gelu(h@blk_w+blk_b + integral)
    out = h @ proj_w + proj_b

Math restructure:
    (q k^T) v == q (k^T v)                    (no softmax)
    k^T v     == kvk^T G kvv,  G = H_aug^T H_aug     ([65,65] Gram)
    layer     == gelu( H @ W64 + b_eff ),  [W64; b_eff] = blkw_aug + C G kvv,
                 C = (qw_aug*s) kvk^T  (host-precomputed)

Layout: the hidden state lives as four "pair tiles" gt_p [128, 512] bf16 per
layer -- partitions 0-63 = features of token chunk p, 64-127 = features of
chunk p+4.  Each update pair is two concurrent matmuls into the two PE
column-group halves (they overlap on the PE!), one [128,512] gelu (bias adds
b_eff, replacing the ones-row trick), and NO copies back to a flat H buffer.
Transposes, the update's moving operand, and proj all read gt tiles directly
at partition base 0 or 64 (PE tile positions).  The Gram's ones column is
pre-seeded in the HT tile buffers.  All matmul operands are bf16 (4x faster
PE than fp32); accumulation stays fp32 in PSUM.  rel err ~1.5e-3 (tol 2e-2).

Sharding: batch 2 -> data-parallel on 2 NeuronCores.  Sequence-sharding
wider would need a per-layer AllReduce (>=20us latency floor) and loses.
"""

import os
import sys

for _p in ("/opt/trn_rl_repo", "/root/.axon_site/_ro/trn_rl_repo"):
    if os.path.isdir(_p) and _p not in sys.path:
        sys.path.append(_p)

import numpy as np

N = 4096          # tokens per batch element (64*64)
D = 64            # hidden
DA = D + 1        # hidden + ones row
L = 4             # layers
B = 2             # batch / cores used
SCALE = (1.0 / np.sqrt(np.float32(D))) / np.float32(N)

# wpack [65, .] column layout (65-partition constants)
DP = DA + 1  # padded HT slot stride (66) -> 4-byte aligned bf16 PSUM slots
OFF_I = 0                    # I65 (square identity)        65
OFF_KVV = OFF_I + DA         # kvv_aug per layer            4*64
OFF_CT = OFF_KVV + 4 * D     # C^T per layer                4*65
OFF_BLK = OFF_CT + 4 * DA    # blkw_aug per layer           4*64
OFF_E65 = OFF_BLK + 4 * D    # unit column e_64             1
WCOLS = OFF_E65 + 1

# wp2 [128, .] column layout (128-partition constants)
W2_I = 0                     # [I64|0 0] stacked twice      66
W2_PROJ = W2_I + DP          # [proj_w; proj_w]             1
W2_LIFT = W2_PROJ + 1        # lift_w^T in rows 0..3        64
W2COLS = W2_LIFT + D

_CACHE = {}


def _build_nc():
    """Build + compile the per-core Bass program (identical on both cores)."""
    import concourse.bass as bass
    import concourse.tile as tile
    from concourse import bacc, mybir

    f32 = mybir.dt.float32
    bf16 = mybir.dt.bfloat16
    ts = bass.ts
    GELU = mybir.ActivationFunctionType.Gelu

    nc = bacc.Bacc("TRN2", target_bir_lowering=False, debug=False, num_devices=B)

    xt_d = nc.dram_tensor("xt", [4, N], bf16, kind="ExternalInput")
    wp_d = nc.dram_tensor("wpack", [DA, WCOLS], bf16, kind="ExternalInput")
    wp2_d = nc.dram_tensor("wpack2", [128, W2COLS], bf16, kind="ExternalInput")
    y_d = nc.dram_tensor("y", [128, 32], f32, kind="ExternalOutput")

    PS = bass.MemorySpace.PSUM

    with tile.TileContext(nc) as tc:
        with (
            tc.tile_pool(name="consts", bufs=1) as consts,
            tc.tile_pool(name="htp", bufs=2) as htp,
            tc.tile_pool(name="small", bufs=2) as small,
            tc.tile_pool(name="gtp", bufs=8) as gtp,
            tc.tile_pool(name="ps_tp", bufs=2, space=PS) as ps_tp,
            tc.tile_pool(name="ps_sm", bufs=3, space=PS) as ps_sm,
            tc.tile_pool(name="ps_up", bufs=3, space=PS) as ps_up,
        ):
            # ---- load inputs into SBUF (parallel DMA queues) -------------
            xt = consts.tile([4, N], bf16, tag="xt")
            nc.sync.dma_start(xt[:], xt_d.ap())
            wp = consts.tile([DA, WCOLS], bf16, tag="wp")
            nc.gpsimd.dma_start(wp[:], wp_d.ap())
            wp2 = consts.tile([128, W2COLS], bf16, tag="wp2")
            nc.scalar.dma_start(wp2[:], wp2_d.ap())

            I65 = wp[:, OFF_I : OFF_I + DA]
            e65 = wp[:, OFF_E65 : OFF_E65 + 1]

            # HT buffers: 32 slots of stride 66 (cols 0-63 = transposed
            # tokens x features, col 64 = ones, col 65 = pad).  The ones
            # columns are seeded once per buffer and never overwritten.
            HTb = []
            for hb in range(2):
                HT = htp.tile([128, 32 * DP], bf16, tag="ht", name=f"ht{hb}")
                nc.vector.memset(
                    HT.rearrange("p (t c) -> p t c", c=DP)[:, :, D : D + 1],
                    1.0)
                HTb.append(HT)

            # ---- lift: pair tiles gt_c = lift_w^T @ [xt_c | xt_{c+4}] ----
            state = []
            for c in range(4):
                lf_ps = ps_up.tile([128, 512], f32, tag="up")
                nc.tensor.matmul(lf_ps[0:D, :], wp2[0:4, W2_LIFT : W2_LIFT + D],
                                 xt[:, ts(c, 512)], start=True, stop=True)
                nc.tensor.matmul(lf_ps[64:128, :], wp2[0:4, W2_LIFT : W2_LIFT + D],
                                 xt[:, ts(c + 4, 512)], start=True, stop=True)
                gt = gtp.tile([128, 512], bf16, tag="gt", name=f"lift{c}")
                if c % 2 == 0:
                    nc.vector.tensor_copy(gt[:], lf_ps[:])
                else:
                    nc.scalar.copy(gt[:], lf_ps[:])
                state.append(gt)

            # ---- layers --------------------------------------------------
            for l in range(L):
                kvv_l = wp[:, OFF_KVV + l * D : OFF_KVV + (l + 1) * D]
                ct_l = wp[:, OFF_CT + l * DA : OFF_CT + (l + 1) * DA]
                blk_l = wp[:, OFF_BLK + l * D : OFF_BLK + (l + 1) * D]
                HT = HTb[l % 2]

                # W64 accumulator [128,64] (W64 duplicated in both halves):
                # starts with blkw rows 0-63, chain below adds (C G kvv).
                w_ps = ps_sm.tile([128, D], f32, tag="sm")
                nc.tensor.matmul(w_ps[0:D, :], I65[:, 0:D], blk_l,
                                 start=True, stop=False)
                nc.tensor.matmul(w_ps[64:128, :], I65[:, 0:D], blk_l,
                                 start=True, stop=False)

                g_ps = ps_sm.tile([DA, DA], f32, tag="sm")

                # transposes: tile t (tokens 128t..) lives in pair tile
                # state[c%4], partition half c//4, where c = t//4.
                def transpose_group(g):
                    tp_ps = ps_tp.tile([128, 4 * DP], bf16, tag="tp")
                    for k in range(4):
                        t = 4 * g + k
                        c = t // 4
                        src = state[c % 4]
                        rb = 64 * (c // 4)
                        nc.tensor.transpose(
                            tp_ps[:, k * DP : k * DP + DP],
                            src[rb : rb + D,
                                128 * (t % 4) : 128 * (t % 4) + 128],
                            wp2[rb : rb + D, W2_I : W2_I + DP])
                    # strided copy: per-slot cols 0-63 only (ones col stays)
                    nc.vector.tensor_copy(
                        HT.rearrange("p (t c) -> p t c", c=DP)
                          [:, 4 * g : 4 * g + 4, 0:D],
                        tp_ps.rearrange("p (t c) -> p t c", c=DP)[:, :, 0:D])

                def g_group(g):
                    for k in range(4):
                        t = 4 * g + k
                        nc.tensor.matmul(
                            g_ps[:], HT[:, t * DP : t * DP + DA],
                            HT[:, t * DP : t * DP + DA],
                            start=(t == 0), stop=(t == 31))

                # Gram groups lag the transposes by 2 so the PSUM->SBUF
                # copies have a full group-time to land.
                for g in range(8):
                    transpose_group(g)
                    if g > 1:
                        g_group(g - 2)
                g_group(6)
                g_group(7)

                # chain: [W64; b_eff] += C @ (G @ kvv)
                g_sb = small.tile([DA, DA], bf16, tag="gsb")
                nc.vector.tensor_copy(g_sb[:], g_ps[:])
                m1_ps = ps_sm.tile([DA, D], f32, tag="sm")
                nc.tensor.matmul(m1_ps[:], g_sb[:], kvv_l, start=True, stop=True)
                m1_sb = small.tile([DA, D], bf16, tag="m1")
                nc.vector.tensor_copy(m1_sb[:], m1_ps[:])
                nc.tensor.matmul(w_ps[0:D, :], ct_l[:, 0:D], m1_sb[:],
                                 start=False, stop=True)
                nc.tensor.matmul(w_ps[64:128, :], ct_l[:, 0:D], m1_sb[:],
                                 start=False, stop=True)
                wupd = small.tile([128, D], bf16, tag="wupd")
                nc.vector.tensor_copy(wupd[:], w_ps[:])
                # bias b_eff = row 64 of (blkw + C G kvv), both halves
                b_ps = ps_sm.tile([128, 1], f32, tag="sm")
                nc.tensor.matmul(b_ps[0:D, :], blk_l, e65,
                                 start=True, stop=False)
                nc.tensor.matmul(b_ps[64:128, :], blk_l, e65,
                                 start=True, stop=False)
                nc.tensor.matmul(b_ps[0:D, :], m1_sb[:], ct_l[:, D : DA],
                                 start=False, stop=True)
                nc.tensor.matmul(b_ps[64:128, :], m1_sb[:], ct_l[:, D : DA],
                                 start=False, stop=True)
                bias = small.tile([128, 1], f32, tag="bias")
                nc.vector.tensor_copy(bias[:], b_ps[:])

                # update: gt'_p = gelu([h_p | h_{p+4}] @ W64 + b_eff); the
                # two matmuls run concurrently in disjoint PE quadrants.
                if l == L - 1:
                    y_ps = ps_sm.tile([128, 32], f32, tag="sm")
                nstate = []
                for p in range(4):
                    up_ps = ps_up.tile([128, 512], f32, tag="up")
                    nc.tensor.matmul(up_ps[0:D, :], wupd[0:D, :],
                                     state[p][0:D, :], start=True, stop=True)
                    nc.tensor.matmul(up_ps[64:128, :], wupd[64:128, :],
                                     state[p][64:128, :], start=True, stop=True)
                    gt = gtp.tile([128, 512], bf16, tag="gt", name=f"gt{l}_{p}")
                    nc.scalar.activation(gt[:], up_ps[:], GELU, bias=bias[:])
                    nstate.append(gt)
                    if l == L - 1:
                        # proj (token-major, y_ps[q, t] = y[128*t + q]);
                        # proj_b is added on the host after gather.
                        for half in range(2):
                            c = p + 4 * half
                            rb = 64 * half
                            for s in range(4):
                                tt = 4 * c + s
                                nc.tensor.matmul(
                                    y_ps[:, tt : tt + 1],
                                    gt[rb : rb + D, 128 * s : 128 * s + 128],
                                    wp2[rb : rb + D, W2_PROJ : W2_PROJ + 1],
                                    start=True, stop=True)
                state = nstate

            # ---- drain y -------------------------------------------------
            out_sb = consts.tile([128, 32], f32, tag="out")
            nc.vector.tensor_copy(out_sb[:], y_ps[:])
            nc.sync.dma_start(y_d.ap(), out_sb[:])

    nc.compile()
    return nc


def _prep_inputs(x, lift_w, lift_b, blk_w, blk_b, q_w, q_b, k_w, k_b, v_w,
                 v_b, proj_w, proj_b):
    """Host-side weight packing (tiny [64,64] reshuffles, negligible cost)."""
    import ml_dtypes
    bf16 = ml_dtypes.bfloat16
    f = lambda a: np.asarray(a, dtype=np.float32)
    x = f(x)
    lift_w, lift_b = f(lift_w), f(lift_b)
    blk_w, blk_b = f(blk_w), f(blk_b)
    q_w, q_b, k_w, k_b, v_w, v_b = f(q_w), f(q_b), f(k_w), f(k_b), f(v_w), f(v_b)
    proj_w, proj_b = f(proj_w), f(proj_b)

    wpack = np.zeros((DA, WCOLS), np.float32)
    wpack[:, OFF_I : OFF_I + DA] = np.eye(DA, dtype=np.float32)
    for l in range(L):
        kvk = np.vstack([k_w[l], k_b[l][None]])            # [65, 64]
        kvv = np.vstack([v_w[l], v_b[l][None]])            # [65, 64]
        qts = (np.vstack([q_w[l], q_b[l][None]]) * SCALE).T  # [64, 65]
        ct = kvk @ qts                                     # [65, 65] = C^T
        wpack[:, OFF_KVV + l * D : OFF_KVV + (l + 1) * D] = kvv
        wpack[:, OFF_CT + l * DA : OFF_CT + (l + 1) * DA] = ct
        wpack[:, OFF_BLK + l * D : OFF_BLK + (l + 1) * D] = \
            np.vstack([blk_w[l], blk_b[l][None]])
    wpack[D, OFF_E65] = 1.0
    wpack = wpack.astype(bf16)

    wpack2 = np.zeros((128, W2COLS), np.float32)
    wpack2[0:D, W2_I : W2_I + D] = np.eye(D, dtype=np.float32)
    wpack2[D : 2 * D, W2_I : W2_I + D] = np.eye(D, dtype=np.float32)
    wpack2[0:D, W2_PROJ] = proj_w[:, 0]
    wpack2[D : 2 * D, W2_PROJ] = proj_w[:, 0]
    wpack2[0:3, W2_LIFT : W2_LIFT + D] = lift_w
    wpack2[3, W2_LIFT : W2_LIFT + D] = lift_b
    wpack2 = wpack2.astype(bf16)

    in_maps = []
    for b in range(B):
        xt = np.concatenate([x[b].reshape(N, 3).T,
                             np.ones((1, N), np.float32)], axis=0).astype(bf16)
        in_maps.append({"xt": np.ascontiguousarray(xt), "wpack": wpack,
                        "wpack2": wpack2})
    return in_maps, x.shape, np.float32(proj_b[0])


def _get_runner():
    """Compile once, return a fn(in_maps) -> list[{name: np.ndarray}]."""
    if "runner" in _CACHE:
        return _CACHE["runner"]

    import jax
    from jax.sharding import Mesh, PartitionSpec
    try:
        from jax.experimental.shard_map import shard_map
    except ImportError:  # newer jax
        from jax.sharding import shard_map
    from concourse import mybir
    from concourse.bass2jax import (_bass_exec_p, install_neuronx_cc_hook,
                                    partition_id_tensor)

    nc = _build_nc()
    install_neuronx_cc_hook()

    partition_name = (nc.partition_id_tensor.name
                      if nc.partition_id_tensor else None)
    in_names, out_names, out_avals, zero_outs = [], [], [], []
    for alloc in nc.m.functions[0].allocations:
        if not isinstance(alloc, mybir.MemoryLocationSet):
            continue
        name = alloc.memorylocations[0].name
        if alloc.kind == "ExternalInput":
            if name != partition_name:
                in_names.append(name)
        elif alloc.kind == "ExternalOutput":
            shape = tuple(alloc.tensor_shape)
            dtype = mybir.dt.np(alloc.dtype)
            out_names.append(name)
            out_avals.append(jax.core.ShapedArray(shape, dtype))
            zero_outs.append(np.zeros(shape, dtype))
    n_params = len(in_names)
    n_outs = len(out_avals)
    all_in_names = in_names + out_names + ([partition_name] if partition_name else [])
    donate = tuple(range(n_params, n_params + n_outs))

    def _body(*args):
        operands = list(args)
        if partition_name is not None:
            operands.append(partition_id_tensor())
        return tuple(_bass_exec_p.bind(
            *operands, out_avals=tuple(out_avals), in_names=tuple(all_in_names),
            out_names=tuple(out_names), lowering_input_output_aliases=(),
            sim_require_finite=True, sim_require_nnan=True, nc=nc))

    devices = jax.devices()[:B]
    mesh = Mesh(np.asarray(devices), ("core",))
    sharded = jax.jit(
        shard_map(_body, mesh=mesh,
                  in_specs=(PartitionSpec("core"),) * (n_params + n_outs),
                  out_specs=(PartitionSpec("core"),) * n_outs,
                  check_rep=False),
        donate_argnums=donate, keep_unused=True)

    def run(in_maps):
        per_core = [[np.asarray(m[name]) for name in in_names] for m in in_maps]
        concat_in = [np.concatenate([per_core[c][i] for c in range(B)], axis=0)
                     for i in range(n_params)]
        big_zeros = [np.concatenate([z] * B, axis=0) for z in zero_outs]
        outs = jax.block_until_ready(sharded(*concat_in, *big_zeros))
        results = []
        for c in range(B):
            r = {}
            for i, name in enumerate(out_names):
                rows = out_avals[i].shape[0]
                r[name] = np.asarray(outs[i][c * rows : (c + 1) * rows])
            results.append(r)
        return results

    _CACHE["runner"] = run
    return run


def kernel(**inputs) -> np.ndarray:
    in_maps, x_shape, proj_b0 = _prep_inputs(**inputs)
    run = _get_runner()
    results = run(in_maps)
    # y_core [128, 32]: element (q, t) = y[128*t + q] (pre proj_b)
    out = np.stack([(results[b]["y"].T + proj_b0)
                    .reshape(x_shape[1], x_shape[2], 1) for b in range(B)])
    return out.astype(np.float32)


# revision 28
# speedup vs baseline: 1.0515x; 1.0048x over previous
"""Trainium2 Bass kernel for the GNO (Galerkin-type linear attention) model.

Reference computation per batch element b (N=4096 tokens, d=64):
    h = x @ lift_w + lift_b
    for each of 4 layers:
        q = h@q_w+q_b ; k = h@k_w+k_b ; v = h@v_w+v_b
        kern     = (q @ k^T) / sqrt(d)          # [N, N], no softmax!
        integral = (kern @ v) / N               # [N, d]
        h        = gelu(h@blk_w+blk_b + integral)
    out = h @ proj_w + proj_b

Math restructure:
    (q k^T) v == q (k^T v)                    (no softmax)
    k^T v     == kvk^T G kvv,  G = H_aug^T H_aug     ([65,65] Gram)
    layer     == gelu( H @ W64 + b_eff ),  [W64; b_eff] = blkw_aug + C G kvv,
                 C = (qw_aug*s) kvk^T  (host-precomputed)

Layout: the hidden state lives as four "pair tiles" gt_p [128, 512] bf16 per
layer -- partitions 0-63 = features of token chunk p, 64-127 = features of
chunk p+4.  Each update pair is two concurrent matmuls into the two PE
column-group halves (they overlap on the PE!), one [128,512] gelu (bias adds
b_eff, replacing the ones-row trick), and NO copies back to a flat H buffer.
Transposes, the update's moving operand, and proj all read gt tiles directly
at partition base 0 or 64 (PE tile positions).  The Gram's ones column is
pre-seeded in the HT tile buffers.  All matmul operands are bf16 (4x faster
PE than fp32); accumulation stays fp32 in PSUM.  rel err ~1.5e-3 (tol 2e-2).

Sharding: batch 2 -> data-parallel on 2 NeuronCores.  Sequence-sharding
wider would need a per-layer AllReduce (>=20us latency floor) and loses.
"""

import os
import sys

for _p in ("/opt/trn_rl_repo", "/root/.axon_site/_ro/trn_rl_repo"):
    if os.path.isdir(_p) and _p not in sys.path:
        sys.path.append(_p)

import numpy as np

N = 4096          # tokens per batch element (64*64)
D = 64            # hidden
DA = D + 1        # hidden + ones row
L = 4             # layers
B = 2             # batch / cores used
SCALE = (1.0 / np.sqrt(np.float32(D))) / np.float32(N)

# wpack [65, .] column layout (65-partition constants)
DP = DA + 1  # padded HT slot stride (66) -> 4-byte aligned bf16 PSUM slots
OFF_I = 0                    # I65 (square identity)        65
OFF_KVV = OFF_I + DA         # kvv_aug per layer            4*64
OFF_CT = OFF_KVV + 4 * D     # C^T per layer                4*65
OFF_BLK = OFF_CT + 4 * DA    # blkw_aug per layer           4*64
OFF_E65 = OFF_BLK + 4 * D    # unit column e_64             1
WCOLS = OFF_E65 + 1

# wp2 [128, .] column layout (128-partition constants)
W2_I = 0                     # [I64|0 0] stacked twice      66
W2_PROJ = W2_I + DP          # [proj_w; proj_w]             1
W2_LIFT = W2_PROJ + 1        # lift_w^T in rows 0..3        64
W2COLS = W2_LIFT + D

_CACHE = {}


def _build_nc():
    """Build + compile the per-core Bass program (identical on both cores)."""
    import concourse.bass as bass
    import concourse.tile as tile
    from concourse import bacc, mybir

    f32 = mybir.dt.float32
    bf16 = mybir.dt.bfloat16
    ts = bass.ts
    GELU = mybir.ActivationFunctionType.Gelu

    nc = bacc.Bacc("TRN2", target_bir_lowering=False, debug=False, num_devices=B)

    xt_d = nc.dram_tensor("xt", [4, N], bf16, kind="ExternalInput")
    wp_d = nc.dram_tensor("wpack", [DA, WCOLS], bf16, kind="ExternalInput")
    wp2_d = nc.dram_tensor("wpack2", [128, W2COLS], bf16, kind="ExternalInput")
    y_d = nc.dram_tensor("y", [128, 32], f32, kind="ExternalOutput")

    PS = bass.MemorySpace.PSUM

    with tile.TileContext(nc) as tc:
        with (
            tc.tile_pool(name="consts", bufs=1) as consts,
            tc.tile_pool(name="htp", bufs=2) as htp,
            tc.tile_pool(name="small", bufs=2) as small,
            tc.tile_pool(name="gtp", bufs=8) as gtp,
            tc.tile_pool(name="ps_tp", bufs=2, space=PS) as ps_tp,
            tc.tile_pool(name="ps_sm", bufs=3, space=PS) as ps_sm,
            tc.tile_pool(name="ps_up", bufs=3, space=PS) as ps_up,
        ):
            # ---- load inputs into SBUF (parallel DMA queues) -------------
            xt = consts.tile([4, N], bf16, tag="xt")
            nc.sync.dma_start(xt[:], xt_d.ap())
            wp = consts.tile([DA, WCOLS], bf16, tag="wp")
            nc.gpsimd.dma_start(wp[:], wp_d.ap())
            wp2 = consts.tile([128, W2COLS], bf16, tag="wp2")
            nc.scalar.dma_start(wp2[:], wp2_d.ap())

            I65 = wp[:, OFF_I : OFF_I + DA]
            e65 = wp[:, OFF_E65 : OFF_E65 + 1]

            # HT buffers: 32 slots of stride 66 (cols 0-63 = transposed
            # tokens x features, col 64 = ones, col 65 = pad).  The ones
            # columns are seeded once per buffer and never overwritten.
            HTb = []
            for hb in range(2):
                HT = htp.tile([128, 32 * DP], bf16, tag="ht", name=f"ht{hb}")
                nc.vector.memset(
                    HT.rearrange("p (t c) -> p t c", c=DP)[:, :, D : D + 1],
                    1.0)
                HTb.append(HT)

            # ---- lift: pair tiles gt_c = lift_w^T @ [xt_c | xt_{c+4}] ----
            state = []
            for c in range(4):
                lf_ps = ps_up.tile([128, 512], f32, tag="up")
                nc.tensor.matmul(lf_ps[0:D, :], wp2[0:4, W2_LIFT : W2_LIFT + D],
                                 xt[:, ts(c, 512)], start=True, stop=True)
                nc.tensor.matmul(lf_ps[64:128, :], wp2[0:4, W2_LIFT : W2_LIFT + D],
                                 xt[:, ts(c + 4, 512)], start=True, stop=True)
                gt = gtp.tile([128, 512], bf16, tag="gt", name=f"lift{c}")
                if c % 2 == 0:
                    nc.vector.tensor_copy(gt[:], lf_ps[:])
                else:
                    nc.scalar.copy(gt[:], lf_ps[:])
                state.append(gt)

            # ---- layers --------------------------------------------------
            for l in range(L):
                kvv_l = wp[:, OFF_KVV + l * D : OFF_KVV + (l + 1) * D]
                ct_l = wp[:, OFF_CT + l * DA : OFF_CT + (l + 1) * DA]
                blk_l = wp[:, OFF_BLK + l * D : OFF_BLK + (l + 1) * D]
                HT = HTb[l % 2]

                # W64 accumulator [128,64] (W64 duplicated in both halves):
                # starts with blkw rows 0-63, chain below adds (C G kvv).
                w_ps = ps_sm.tile([128, D], f32, tag="sm")
                nc.tensor.matmul(w_ps[0:D, :], I65[:, 0:D], blk_l,
                                 start=True, stop=False)
                nc.tensor.matmul(w_ps[64:128, :], I65[:, 0:D], blk_l,
                                 start=True, stop=False)

                g_ps = ps_sm.tile([DA, DA], f32, tag="sm")

                # transposes: tile t (tokens 128t..) lives in pair tile
                # state[c%4], partition half c//4, where c = t//4.
                def transpose_group(g):
                    tp_ps = ps_tp.tile([128, 4 * DP], bf16, tag="tp")
                    for k in range(4):
                        t = 4 * g + k
                        c = t // 4
                        src = state[c % 4]
                        rb = 64 * (c // 4)
                        nc.tensor.transpose(
                            tp_ps[:, k * DP : k * DP + DP],
                            src[rb : rb + D,
                                128 * (t % 4) : 128 * (t % 4) + 128],
                            wp2[rb : rb + D, W2_I : W2_I + DP])
                    # strided copy, slot cols 0-63 only (ones col stays);
                    # bf16 pairs viewed as uint32 halve the DVE element
                    # count (PSUM bf16 is densely packed)
                    u32 = mybir.dt.uint32
                    nc.vector.tensor_copy(
                        HT.bitcast(u32)
                          .rearrange("p (t c) -> p t c", c=DP // 2)
                          [:, 4 * g : 4 * g + 4, 0 : D // 2],
                        tp_ps.bitcast(u32)
                             .rearrange("p (t c) -> p t c", c=DP // 2)
                             [:, :, 0 : D // 2])

                def g_group(g):
                    for k in range(4):
                        t = 4 * g + k
                        nc.tensor.matmul(
                            g_ps[:], HT[:, t * DP : t * DP + DA],
                            HT[:, t * DP : t * DP + DA],
                            start=(t == 0), stop=(t == 31))

                # Gram groups lag the transposes by 2 so the PSUM->SBUF
                # copies have a full group-time to land.
                for g in range(8):
                    transpose_group(g)
                    if g > 1:
                        g_group(g - 2)
                g_group(6)
                g_group(7)

                # chain: [W64; b_eff] += C @ (G @ kvv)
                g_sb = small.tile([DA, DA], bf16, tag="gsb")
                nc.vector.tensor_copy(g_sb[:], g_ps[:])
                m1_ps = ps_sm.tile([DA, D], f32, tag="sm")
                nc.tensor.matmul(m1_ps[:], g_sb[:], kvv_l, start=True, stop=True)
                m1_sb = small.tile([DA, D], bf16, tag="m1")
                nc.vector.tensor_copy(m1_sb[:], m1_ps[:])
                nc.tensor.matmul(w_ps[0:D, :], ct_l[:, 0:D], m1_sb[:],
                                 start=False, stop=True)
                nc.tensor.matmul(w_ps[64:128, :], ct_l[:, 0:D], m1_sb[:],
                                 start=False, stop=True)
                wupd = small.tile([128, D], bf16, tag="wupd")
                nc.vector.tensor_copy(wupd[:], w_ps[:])
                # bias b_eff = row 64 of (blkw + C G kvv), both halves
                b_ps = ps_sm.tile([128, 1], f32, tag="sm")
                nc.tensor.matmul(b_ps[0:D, :], blk_l, e65,
                                 start=True, stop=False)
                nc.tensor.matmul(b_ps[64:128, :], blk_l, e65,
                                 start=True, stop=False)
                nc.tensor.matmul(b_ps[0:D, :], m1_sb[:], ct_l[:, D : DA],
                                 start=False, stop=True)
                nc.tensor.matmul(b_ps[64:128, :], m1_sb[:], ct_l[:, D : DA],
                                 start=False, stop=True)
                bias = small.tile([128, 1], f32, tag="bias")
                nc.vector.tensor_copy(bias[:], b_ps[:])

                # update: gt'_p = gelu([h_p | h_{p+4}] @ W64 + b_eff); the
                # two matmuls run concurrently in disjoint PE quadrants.
                if l == L - 1:
                    y_ps = ps_sm.tile([128, 32], f32, tag="sm")
                nstate = []
                for p in range(4):
                    up_ps = ps_up.tile([128, 512], f32, tag="up")
                    nc.tensor.matmul(up_ps[0:D, :], wupd[0:D, :],
                                     state[p][0:D, :], start=True, stop=True)
                    nc.tensor.matmul(up_ps[64:128, :], wupd[64:128, :],
                                     state[p][64:128, :], start=True, stop=True)
                    gt = gtp.tile([128, 512], bf16, tag="gt", name=f"gt{l}_{p}")
                    nc.scalar.activation(gt[:], up_ps[:], GELU, bias=bias[:])
                    nstate.append(gt)
                    if l == L - 1:
                        # proj (token-major, y_ps[q, t] = y[128*t + q]);
                        # proj_b is added on the host after gather.
                        for half in range(2):
                            c = p + 4 * half
                            rb = 64 * half
                            for s in range(4):
                                tt = 4 * c + s
                                nc.tensor.matmul(
                                    y_ps[:, tt : tt + 1],
                                    gt[rb : rb + D, 128 * s : 128 * s + 128],
                                    wp2[rb : rb + D, W2_PROJ : W2_PROJ + 1],
                                    start=True, stop=True)
                state = nstate

            # ---- drain y -------------------------------------------------
            out_sb = consts.tile([128, 32], f32, tag="out")
            nc.vector.tensor_copy(out_sb[:], y_ps[:])
            nc.sync.dma_start(y_d.ap(), out_sb[:])

    nc.compile()
    return nc


def _prep_inputs(x, lift_w, lift_b, blk_w, blk_b, q_w, q_b, k_w, k_b, v_w,
                 v_b, proj_w, proj_b):
    """Host-side weight packing (tiny [64,64] reshuffles, negligible cost)."""
    import ml_dtypes
    bf16 = ml_dtypes.bfloat16
    f = lambda a: np.asarray(a, dtype=np.float32)
    x = f(x)
    lift_w, lift_b = f(lift_w), f(lift_b)
    blk_w, blk_b = f(blk_w), f(blk_b)
    q_w, q_b, k_w, k_b, v_w, v_b = f(q_w), f(q_b), f(k_w), f(k_b), f(v_w), f(v_b)
    proj_w, proj_b = f(proj_w), f(proj_b)

    wpack = np.zeros((DA, WCOLS), np.float32)
    wpack[:, OFF_I : OFF_I + DA] = np.eye(DA, dtype=np.float32)
    for l in range(L):
        kvk = np.vstack([k_w[l], k_b[l][None]])            # [65, 64]
        kvv = np.vstack([v_w[l], v_b[l][None]])            # [65, 64]
        qts = (np.vstack([q_w[l], q_b[l][None]]) * SCALE).T  # [64, 65]
        ct = kvk @ qts                                     # [65, 65] = C^T
        wpack[:, OFF_KVV + l * D : OFF_KVV + (l + 1) * D] = kvv
        wpack[:, OFF_CT + l * DA : OFF_CT + (l + 1) * DA] = ct
        wpack[:, OFF_BLK + l * D : OFF_BLK + (l + 1) * D] = \
            np.vstack([blk_w[l], blk_b[l][None]])
    wpack[D, OFF_E65] = 1.0
    wpack = wpack.astype(bf16)

    wpack2 = np.zeros((128, W2COLS), np.float32)
    wpack2[0:D, W2_I : W2_I + D] = np.eye(D, dtype=np.float32)
    wpack2[D : 2 * D, W2_I : W2_I + D] = np.eye(D, dtype=np.float32)
    wpack2[0:D, W2_PROJ] = proj_w[:, 0]
    wpack2[D : 2 * D, W2_PROJ] = proj_w[:, 0]
    wpack2[0:3, W2_LIFT : W2_LIFT + D] = lift_w
    wpack2[3, W2_LIFT : W2_LIFT + D] = lift_b
    wpack2 = wpack2.astype(bf16)

    in_maps = []
    for b in range(B):
        xt = np.concatenate([x[b].reshape(N, 3).T,
                             np.ones((1, N), np.float32)], axis=0).astype(bf16)
        in_maps.append({"xt": np.ascontiguousarray(xt), "wpack": wpack,
                        "wpack2": wpack2})
    return in_maps, x.shape, np.float32(proj_b[0])


def _get_runner():
    """Compile once, return a fn(in_maps) -> list[{name: np.ndarray}]."""
    if "runner" in _CACHE:
        return _CACHE["runner"]

    import jax
    from jax.sharding import Mesh, PartitionSpec
    try:
        from jax.experimental.shard_map import shard_map
    except ImportError:  # newer jax
        from jax.sharding import shard_map
    from concourse import mybir
    from concourse.bass2jax import (_bass_exec_p, install_neuronx_cc_hook,
                                    partition_id_tensor)

    nc = _build_nc()
    install_neuronx_cc_hook()

    partition_name = (nc.partition_id_tensor.name
                      if nc.partition_id_tensor else None)
    in_names, out_names, out_avals, zero_outs = [], [], [], []
    for alloc in nc.m.functions[0].allocations:
        if not isinstance(alloc, mybir.MemoryLocationSet):
            continue
        name = alloc.memorylocations[0].name
        if alloc.kind == "ExternalInput":
            if name != partition_name:
                in_names.append(name)
        elif alloc.kind == "ExternalOutput":
            shape = tuple(alloc.tensor_shape)
            dtype = mybir.dt.np(alloc.dtype)
            out_names.append(name)
            out_avals.append(jax.core.ShapedArray(shape, dtype))
            zero_outs.append(np.zeros(shape, dtype))
    n_params = len(in_names)
    n_outs = len(out_avals)
    all_in_names = in_names + out_names + ([partition_name] if partition_name else [])
    donate = tuple(range(n_params, n_params + n_outs))

    def _body(*args):
        operands = list(args)
        if partition_name is not None:
            operands.append(partition_id_tensor())
        return tuple(_bass_exec_p.bind(
            *operands, out_avals=tuple(out_avals), in_names=tuple(all_in_names),
            out_names=tuple(out_names), lowering_input_output_aliases=(),
            sim_require_finite=True, sim_require_nnan=True, nc=nc))

    devices = jax.devices()[:B]
    mesh = Mesh(np.asarray(devices), ("core",))
    sharded = jax.jit(
        shard_map(_body, mesh=mesh,
                  in_specs=(PartitionSpec("core"),) * (n_params + n_outs),
                  out_specs=(PartitionSpec("core"),) * n_outs,
                  check_rep=False),
        donate_argnums=donate, keep_unused=True)

    def run(in_maps):
        per_core = [[np.asarray(m[name]) for name in in_names] for m in in_maps]
        concat_in = [np.concatenate([per_core[c][i] for c in range(B)], axis=0)
                     for i in range(n_params)]
        big_zeros = [np.concatenate([z] * B, axis=0) for z in zero_outs]
        outs = jax.block_until_ready(sharded(*concat_in, *big_zeros))
        results = []
        for c in range(B):
            r = {}
            for i, name in enumerate(out_names):
                rows = out_avals[i].shape[0]
                r[name] = np.asarray(outs[i][c * rows : (c + 1) * rows])
            results.append(r)
        return results

    _CACHE["runner"] = run
    return run


def kernel(**inputs) -> np.ndarray:
    in_maps, x_shape, proj_b0 = _prep_inputs(**inputs)
    run = _get_runner()
    results = run(in_maps)
    # y_core [128, 32]: element (q, t) = y[128*t + q] (pre proj_b)
    out = np.stack([(results[b]["y"].T + proj_b0)
                    .reshape(x_shape[1], x_shape[2], 1) for b in range(B)])
    return out.astype(np.float32)
